# revision 1
# baseline (speedup 1.0000x reference)
"""BondBreakGNN Trainium2 kernel: 2-layer GCN + edge MLP over 100K nodes / 1.6M edges.

Strategy (edge-parallel across 8 cores, dst-sharded):
  - Edges (plus one self-edge per node) are sorted by destination block (128
    nodes) and grouped by source quartile (int16 gather index limit), padded
    to 128-multiples with a layout that is uniform across cores.
  - Scatter-add is computed as one-hot matmuls on the tensor engine: for each
    128-edge subchunk an S[e,v] = (dstoff_e == v) selector (DVE is_equal) is
    contracted with the gathered messages, accumulating per-block into PSUM.
  - Layer 1 messages are built on-device from host-pre-gathered x rows
    (feature-major bf16) via a W1 matmul; dinv[src] is folded into the
    selector build. Layers 2/3 use dma_gather (4 SWDGE queues) from
    AllGather'd per-node tensors.
  - Edge MLP: z = relu(W3a.T h2[src] + u2[dst] + W3c.T attr + b3), with
    u2 = h2_block @ W3b precomputed per block and routed through the same
    one-hot selector; out = W4.T z + b4.
"""
import sys

sys.path.insert(0, "/opt/trn_rl_repo")

import numpy as np
import ml_dtypes

import concourse.bacc as bacc
import concourse.mybir as mybir
import concourse.tile as tile
from concourse.masks import make_identity
from concourse.bass_utils import run_bass_kernel_spmd

BF16 = ml_dtypes.bfloat16


class Cfg:
    def __init__(self, n_nodes, feat=128, hid=64, cores=8, qr=32768, blk_per_core=None):
        self.N = n_nodes
        self.FEAT = feat
        self.HID = hid
        self.CORES = cores
        self.QR = qr  # gather index range per source region (int16 limit)
        if blk_per_core is None:
            blk_per_core = (n_nodes + 128 * cores - 1) // (128 * cores)
        self.BLK = blk_per_core
        self.NPC = 128 * blk_per_core          # nodes per core (padded)
        self.NPAD = self.NPC * cores           # padded node count
        self.NQ = (self.NPAD + qr - 1) // qr   # number of source regions


PROD = Cfg(100000)


def _wrap_idx16(idx):
    """idx [n] -> [128, n//16] int16 wrapped in 16 partitions, replicated x8."""
    n = idx.shape[0]
    w = idx.astype(np.int16).reshape(n // 16, 16).T
    return np.tile(w, (8, 1))


def host_prep(cfg, x, edge_index, edge_attr):
    N, NPAD, QR, CORES, BLK, NPC = cfg.N, cfg.NPAD, cfg.QR, cfg.CORES, cfg.BLK, cfg.NPC
    E = edge_index.shape[1]
    src = np.asarray(edge_index[0], dtype=np.int64)
    dst = np.asarray(edge_index[1], dtype=np.int64)
    attr = np.asarray(edge_attr, dtype=np.float32).reshape(-1)

    # augment with self edges for every (padded) node
    loop = np.arange(NPAD, dtype=np.int64)
    src_a = np.concatenate([src, loop])
    dst_a = np.concatenate([dst, loop])
    attr_a = np.concatenate([attr, np.zeros(NPAD, np.float32)])
    orig_a = np.concatenate([np.arange(E, dtype=np.int64), np.full(NPAD, -1, np.int64)])

    deg = np.bincount(dst, minlength=NPAD).astype(np.float32) + 1.0

    block = dst_a >> 7
    q = src_a // QR
    NQ = cfg.NQ
    key = block * NQ + q
    order = np.argsort(key, kind="stable")
    key_s = key[order]
    cnt = np.bincount(key, minlength=CORES * BLK * NQ).reshape(CORES, BLK, NQ)
    gmax = cnt.max(axis=0)  # [BLK, NQ]
    G = ((gmax + 127) // 128) * 128
    G[gmax == 0] = 0
    S_blk = G.sum(axis=1)          # slots per block position
    S = int(S_blk.sum())           # slots per core
    maxblk = int(S_blk.max())
    # start offset of each group within the slot array
    gflat = G.reshape(-1)
    goff = np.zeros(BLK * NQ, np.int64)
    goff[1:] = np.cumsum(gflat)[:-1]
    # start of each (core, blockpos, q) run in `order`
    kstart = np.zeros(CORES * BLK * NQ + 1, np.int64)
    kstart[1:] = np.cumsum(np.bincount(key, minlength=CORES * BLK * NQ))

    x_b = np.ascontiguousarray(x.astype(BF16))

    per_core = []
    for c in range(CORES):
        slot_src = np.zeros(S, np.int64)            # global src (for x pre-gather)
        slot_rel = np.zeros(S, np.int64)            # src - QR*q (device gather idx)
        slot_dstoff = np.full(S, 255, np.float32)
        slot_attr = np.zeros(S, np.float32)
        slot_orig = np.full(S, -1, np.int64)
        for i in range(BLK):
            for qq in range(NQ):
                g = int(G[i, qq])
                if g == 0:
                    continue
                kk = (c * BLK + i) * NQ + qq
                a, b = kstart[kk], kstart[kk + 1]
                n = b - a
                e = order[a:b]
                o = int(goff[i * NQ + qq])
                slot_src[o:o + n] = src_a[e]
                slot_rel[o:o + n] = src_a[e] - QR * qq
                slot_rel[o + n:o + g] = 0
                slot_dstoff[o:o + n] = (dst_a[e] - 128 * (c * BLK + i)).astype(np.float32)
                slot_attr[o:o + n] = attr_a[e]
                slot_orig[o:o + n] = orig_a[e]

        # x pre-gather, feature-major bf16 [FEAT, S]; rows >= N are zero
        xs = np.zeros((S, cfg.FEAT), BF16)
        valid = slot_src < N
        xs[valid] = x_b[slot_src[valid]]
        xg = np.ascontiguousarray(xs.T)

        idx16 = np.concatenate(
            [_wrap_idx16(slot_rel[int(goff[i * NQ + qq]):int(goff[i * NQ + qq]) + int(G[i, qq])])
             for i in range(BLK) for qq in range(NQ) if G[i, qq] > 0],
            axis=1)

        dstoff_col = np.ascontiguousarray(slot_dstoff.reshape(S // 128, 128).T)  # [128, S/128] f32
        degsrc = deg[np.minimum(slot_src, NPAD - 1)].astype(np.float32)
        degsrc_col = np.ascontiguousarray(degsrc.reshape(S // 128, 128).T)
        slot_dst = 128 * (c * BLK) + np.zeros(S, np.int64)
        boff2 = 0
        for i in range(BLK):
            sb2 = int(S_blk[i])
            dd = slot_dstoff[boff2:boff2 + sb2]
            slot_dst[boff2:boff2 + sb2] = np.where(
                dd < 128, 128 * (c * BLK + i) + dd.astype(np.int64), 0)
            boff2 += sb2
        degdst = deg[slot_dst].astype(np.float32)
        degdst_col = np.ascontiguousarray(degdst.reshape(S // 128, 128).T)
        dstrep = np.ascontiguousarray(
            np.broadcast_to(slot_dstoff.astype(BF16), (128, S)))

        dstoff_row = np.full((BLK, maxblk), 255.0, np.float32)
        attr_row = np.zeros((BLK, maxblk), np.float32)
        boff = 0
        for i in range(BLK):
            sb = int(S_blk[i])
            dstoff_row[i, :sb] = slot_dstoff[boff:boff + sb]
            attr_row[i, :sb] = slot_attr[boff:boff + sb]
            boff += sb

        per_core.append(dict(
            xg=xg, idx16=idx16, dstoff_col=dstoff_col, degsrc_col=degsrc_col,
            degdst_col=degdst_col, dstrep=dstrep,
            attr_row=attr_row.astype(BF16), slot_orig=slot_orig,
        ))

    layout = dict(G=G, S_blk=S_blk, S=S, maxblk=maxblk, goff=goff)
    return per_core, layout


def build_program(cfg, layout, weights):
    """weights: dict W1..b4 as numpy (cast host-side to the dtypes below)."""
    G, S_blk, S, maxblk = layout["G"], layout["S_blk"], layout["S"], layout["maxblk"]
    goff = layout["goff"]
    BLK, NQ, QR, NPAD, HID, FEAT = cfg.BLK, cfg.NQ, cfg.QR, cfg.NPAD, cfg.HID, cfg.FEAT
    f32, bf, i16 = mybir.dt.float32, mybir.dt.bfloat16, mybir.dt.int16

    nc = bacc.Bacc("TRN2", target_bir_lowering=False, debug=False,
                   num_devices=cfg.CORES, num_swdge_queues=4,
                   dynamic_dma_scratch_size=131072)

    # ---- I/O
    xg_d = nc.dram_tensor("xg", [FEAT, S], bf, kind="ExternalInput")
    idx_d = nc.dram_tensor("idx16", [128, S // 16], i16, kind="ExternalInput")
    dstc_d = nc.dram_tensor("dstoff_col", [128, S // 128], f32, kind="ExternalInput")
    degs_d = nc.dram_tensor("degsrc_col", [128, S // 128], f32, kind="ExternalInput")
    degd_d = nc.dram_tensor("degdst_col", [128, S // 128], f32, kind="ExternalInput")
    dstrep_d = nc.dram_tensor("dstrep", [128, S], bf, kind="ExternalInput")
    attr_d = nc.dram_tensor("attr_row", [BLK, maxblk], bf, kind="ExternalInput")
    W1_d = nc.dram_tensor("W1", [FEAT, HID], bf, kind="ExternalInput")
    W2_d = nc.dram_tensor("W2", [HID, HID], bf, kind="ExternalInput")
    W3a_d = nc.dram_tensor("W3a", [128, HID], bf, kind="ExternalInput")  # rows 64.. zero
    W3b_d = nc.dram_tensor("W3b", [HID, HID], bf, kind="ExternalInput")
    W3c_d = nc.dram_tensor("W3c", [1, HID], bf, kind="ExternalInput")
    W4_d = nc.dram_tensor("W4", [HID, 1], bf, kind="ExternalInput")
    b1_d = nc.dram_tensor("b1", [HID, 1], f32, kind="ExternalInput")
    b2_d = nc.dram_tensor("b2", [HID, 1], f32, kind="ExternalInput")
    b3_d = nc.dram_tensor("b3", [HID, 1], f32, kind="ExternalInput")
    b4_d = nc.dram_tensor("b4", [1, 1], f32, kind="ExternalInput")
    zout_d = nc.dram_tensor("zout", [BLK, maxblk], f32, kind="ExternalOutput")

    NPC = cfg.NPC
    relu = mybir.ActivationFunctionType.Relu
    copyf = mybir.ActivationFunctionType.Copy
    identf = mybir.ActivationFunctionType.Identity
    EQ, MUL = mybir.AluOpType.is_equal, mybir.AluOpType.mult

    with tile.TileContext(nc) as tc:
        with (
            tc.tile_pool(name="res", bufs=1) as res,
            tc.tile_pool(name="dram", bufs=1, space="DRAM") as dram,
        ):
            # ---------- resident tiles / constants
            iota16 = res.tile([128, 128], i16)
            nc.gpsimd.iota(iota16[:], pattern=[[1, 128]], base=0, channel_multiplier=0)
            iota_bf = res.tile([128, 128], bf)
            nc.vector.tensor_copy(iota_bf[:], iota16[:])
            iotac16 = res.tile([128, 1], i16)
            nc.gpsimd.iota(iotac16[:], pattern=[[1, 1]], base=0, channel_multiplier=1)
            iota_col = res.tile([128, 1], f32)
            nc.vector.tensor_copy(iota_col[:], iotac16[:])
            ident = res.tile([128, 128], f32)
            make_identity(nc, ident[:])
            ident_bf = res.tile([128, 128], bf)
            nc.vector.tensor_copy(ident_bf[:], ident[:])

            dstc = res.tile([128, S // 128], f32)
            nc.sync.dma_start(dstc[:], dstc_d[:, :])

            W1 = res.tile([FEAT, HID], bf)
            nc.sync.dma_start(W1[:], W1_d[:, :])
            W2 = res.tile([HID, HID], bf)
            nc.sync.dma_start(W2[:], W2_d[:, :])
            W3a = res.tile([128, HID], bf)
            nc.sync.dma_start(W3a[:], W3a_d[:, :])
            W3b = res.tile([HID, HID], bf)
            nc.sync.dma_start(W3b[:], W3b_d[:, :])
            W3c = res.tile([1, HID], bf)
            nc.sync.dma_start(W3c[:], W3c_d[:, :])
            W4 = res.tile([HID, 1], bf)
            nc.sync.dma_start(W4[:], W4_d[:, :])
            b1 = res.tile([HID, 1], f32)
            nc.sync.dma_start(b1[:], b1_d[:, :])
            b2 = res.tile([HID, 1], f32)
            nc.sync.dma_start(b2[:], b2_d[:, :])
            b3 = res.tile([HID, 1], f32)
            nc.sync.dma_start(b3[:], b3_d[:, :])
            b4 = res.tile([1, 1], f32)
            nc.sync.dma_start(b4[:], b4_d[:, :])

            # per-slot norm = 1/sqrt(deg[src]*deg[dst])
            normc = res.tile([128, S // 128], f32)
            with tc.tile_pool(name="degtmp", bufs=1) as degtmp:
                dgs = degtmp.tile([128, S // 128], f32)
                nc.sync.dma_start(dgs[:], degs_d[:, :])
                dgd = degtmp.tile([128, S // 128], f32)
                nc.sync.dma_start(dgd[:], degd_d[:, :])
                nc.vector.tensor_tensor(normc[:], dgs[:], dgd[:], op=MUL)
                nc.scalar.sqrt(normc[:], normc[:])
                nc.vector.reciprocal(normc[:], normc[:])

            u2T_all = res.tile([128, BLK * HID], bf)

            g2_sh = dram.tile([NPC, 128], bf)
            g2_full = dram.tile([NPAD, 128], bf)
            h2_sh = dram.tile([NPC, 128], bf)
            h2_full = dram.tile([NPAD, 128], bf)

            with (
                tc.tile_pool(name="ps_s", bufs=2, space="PSUM") as ps_s,
                tc.tile_pool(name="ps_b", bufs=2, space="PSUM") as ps_b,
                tc.tile_pool(name="ps_a", bufs=2, space="PSUM") as ps_a,
                tc.tile_pool(name="sb", bufs=2) as sb,
                tc.tile_pool(name="sb2", bufs=2) as sb2,
            ):
                qcall = 0

                # ================= Layer 1 =================
                for i in range(BLK):
                    sblk = int(S_blk[i])
                    if sblk == 0:
                        continue
                    nsub = sblk // 128
                    base = int(goff[i * NQ])  # slot offset of block start
                    # stream x-gathered block [FEAT, sblk]
                    xgt = sb.tile([FEAT, maxblk], bf, tag="big")
                    nc.sync.dma_start(xgt[:, :sblk], xg_d[:, base:base + sblk])
                    # expansion: M1[e,hid] per subchunk, batched 8 per PSUM bank
                    m1 = sb.tile([128, (maxblk // 128) * HID], bf, tag="med")
                    for j0 in range(0, nsub, 8):
                        jn = min(8, nsub - j0)
                        mp = ps_b.tile([128, 512], f32, tag="m1p")
                        for j in range(j0, j0 + jn):
                            nc.tensor.matmul(
                                mp[:, (j - j0) * HID:(j - j0 + 1) * HID],
                                xgt[:, j * 128:(j + 1) * 128], W1[:],
                                start=True, stop=True)
                        nc.scalar.activation(
                            m1[:, j0 * HID:(j0 + jn) * HID], mp[:, :jn * HID], copyf)
                    # scatter: agg[hid, 128v]
                    aggp = ps_a.tile([HID, 128], f32, tag="agg")
                    for j in range(nsub):
                        st = sb.tile([128, 128], bf, tag="st")
                        col = base // 128 + j
                        nc.vector.tensor_scalar(
                            st[:], iota_bf[:], dstc[:, col:col + 1],
                            normc[:, col:col + 1], op0=EQ, op1=MUL)
                        nc.tensor.matmul(aggp[:], m1[:, j * HID:(j + 1) * HID], st[:],
                                         start=(j == 0), stop=(j == nsub - 1))
                    h1T = sb2.tile([HID, 128], bf, tag="h1T")
                    nc.scalar.activation(h1T[:], aggp[:], relu, bias=b1[:])
                    # g2 = dinv * (h1 @ W2), node-major
                    g2p = ps_b.tile([128, HID], f32, tag="small")
                    nc.tensor.matmul(g2p[:], h1T[:], W2[:], start=True, stop=True)
                    g2sb = sb2.tile([128, HID], bf, tag="g2sb")
                    nc.scalar.activation(g2sb[:], g2p[:], copyf)
                    nc.sync.dma_start(g2_sh[i * 128:(i + 1) * 128, 0:HID], g2sb[:])

                nc.gpsimd.collective_compute(
                    "AllGather", mybir.AluOpType.bypass,
                    replica_groups=[list(range(cfg.CORES))],
                    ins=[g2_sh[:].opt()], outs=[g2_full[:].opt()])

                # ================= Layer 2 =================
                for i in range(BLK):
                    sblk = int(S_blk[i])
                    if sblk == 0:
                        continue
                    nsub = sblk // 128
                    base = int(goff[i * NQ])
                    mg = sb.tile([128, maxblk // 128, 128], bf, tag="big")
                    idxb = sb.tile([128, maxblk // 16], i16, tag="idxb")
                    nc.sync.dma_start(idxb[:, :sblk // 16],
                                      idx_d[:, base // 16:(base + sblk) // 16])
                    for qq in range(NQ):
                        g = int(G[i, qq])
                        if g == 0:
                            continue
                        o = int(goff[i * NQ + qq]) - base
                        lo = QR * qq
                        hi = min(QR * (qq + 1), NPAD)
                        for o2 in range(0, g, 768):
                            gs = min(768, g - o2)
                            oo = o + o2
                            nc.gpsimd.dma_gather(
                                mg[:, oo // 128:(oo + gs) // 128, :],
                                g2_full[lo:hi, :],
                                idxb[:, (o + o2) // 16:(o + o2 + gs) // 16],
                                gs, gs, 128, queue_num=qcall % 4, single_packet=True)
                            qcall += 1
                    aggp = ps_a.tile([HID, 128], f32, tag="agg")
                    for j in range(nsub):
                        st = sb.tile([128, 128], bf, tag="st")
                        col = base // 128 + j
                        nc.vector.tensor_scalar(
                            st[:], iota_bf[:], dstc[:, col:col + 1],
                            normc[:, col:col + 1], op0=EQ, op1=MUL)
                        nc.tensor.matmul(aggp[:], mg[:, j, 0:HID], st[:],
                                         start=(j == 0), stop=(j == nsub - 1))
                    h2T = sb2.tile([HID, 128], bf, tag="h2T")
                    nc.scalar.activation(h2T[:], aggp[:], relu, bias=b2[:])
                    # u2T[v,j] = (h2_block @ W3b)
                    u2p = ps_b.tile([128, HID], f32, tag="small")
                    nc.tensor.matmul(u2p[:], h2T[:], W3b[:], start=True, stop=True)
                    nc.scalar.activation(u2T_all[:, i * HID:(i + 1) * HID], u2p[:], copyf)
                    # h2 node-major -> HBM
                    trp = ps_b.tile([128, HID], bf, tag="small")
                    nc.tensor.transpose(trp[:], h2T[:], ident_bf[:HID, :HID])
                    h2n = sb2.tile([128, 128], bf, tag="h2n")
                    nc.vector.memset(h2n[:, HID:], 0)
                    nc.vector.tensor_copy(h2n[:, 0:HID], trp[:])
                    nc.sync.dma_start(h2_sh[i * 128:(i + 1) * 128, :], h2n[:])

                nc.gpsimd.collective_compute(
                    "AllGather", mybir.AluOpType.bypass,
                    replica_groups=[list(range(cfg.CORES))],
                    ins=[h2_sh[:].opt()], outs=[h2_full[:].opt()])

                # ================= Layer 3 (edge MLP) =================
                for i in range(BLK):
                    sblk = int(S_blk[i])
                    if sblk == 0:
                        continue
                    base = int(goff[i * NQ])
                    zrow = sb2.tile([1, maxblk], f32, tag="zrow")
                    dsr = sb.tile([128, maxblk], bf, tag="med")
                    nc.sync.dma_start(dsr[:, :sblk], dstrep_d[:, base:base + sblk])
                    hs = sb.tile([128, 1, maxblk], bf, tag="big")
                    idxb = sb.tile([128, maxblk // 16], i16, tag="idxb")
                    nc.sync.dma_start(idxb[:, :sblk // 16],
                                      idx_d[:, base // 16:(base + sblk) // 16])
                    for qq in range(NQ):
                        g = int(G[i, qq])
                        if g == 0:
                            continue
                        o = int(goff[i * NQ + qq]) - base
                        lo = QR * qq
                        hi = min(QR * (qq + 1), NPAD)
                        for o2 in range(0, g, 768):
                            gs = min(768, g - o2)
                            oo = o + o2
                            nc.gpsimd.dma_gather(
                                hs[:, :, oo:oo + gs],
                                h2_full[lo:hi, :],
                                idxb[:, (o + o2) // 16:(o + o2 + gs) // 16],
                                gs, gs, 128, transpose=True,
                                queue_num=qcall % 4, single_packet=True)
                            qcall += 1
                    attrb = sb.tile([1, maxblk], bf, tag="attrb")
                    nc.sync.dma_start(attrb[0:1, :sblk], attr_d[i:i + 1, :sblk])
                    for c0 in range(0, sblk, 512):
                        csz = min(512, sblk - c0)
                        sn = sb.tile([128, 512], bf, tag="sn")
                        nc.vector.tensor_scalar(
                            sn[:, :csz], dsr[:, c0:c0 + csz],
                            iota_col[:], None, op0=EQ)
                        zp = ps_a.tile([HID, 512], f32, tag="zp")
                        nc.tensor.matmul(zp[:, :csz], W3a[:], hs[:, 0, c0:c0 + csz],
                                         start=True, stop=False)
                        nc.tensor.matmul(zp[:, :csz], u2T_all[:, i * HID:(i + 1) * HID],
                                         sn[:, :csz], start=False, stop=False)
                        nc.tensor.matmul(zp[:, :csz], W3c[:],
                                         attrb[0:1, c0:c0 + csz],
                                         start=False, stop=True)
                        zr = sb.tile([HID, 512], bf, tag="zr")
                        nc.scalar.activation(zr[:, :csz], zp[:, :csz], relu, bias=b3[:])
                        op = ps_b.tile([1, 512], f32, tag="small")
                        nc.tensor.matmul(op[:, :csz], W4[:], zr[:, :csz],
                                         start=True, stop=True)
                        nc.scalar.activation(zrow[0:1, c0:c0 + csz], op[:, :csz],
                                             identf, bias=b4[:])
                    nc.sync.dma_start(zout_d[i:i + 1, 0:sblk], zrow[0:1, 0:sblk])

    nc.compile()
    return nc


def _run(cfg, x, edge_index, edge_attr, W1, b1, W2, b2, W3, b3, W4, b4, trace=False):
    per_core, layout = host_prep(cfg, x, edge_index, edge_attr)
    HID = cfg.HID
    W3a = np.zeros((128, HID), BF16)
    W3a[:HID] = W3[:HID].astype(BF16)
    weights = None
    nc = build_program(cfg, layout, weights)

    in_maps = []
    for pc in per_core:
        in_maps.append({
            "xg": pc["xg"], "idx16": pc["idx16"], "dstoff_col": pc["dstoff_col"],
            "degsrc_col": pc["degsrc_col"], "degdst_col": pc["degdst_col"],
            "dstrep": pc["dstrep"], "attr_row": pc["attr_row"],
            "W1": W1.astype(BF16), "W2": W2.astype(BF16),
            "W3a": W3a, "W3b": W3[HID:2 * HID].astype(BF16),
            "W3c": W3[2 * HID:2 * HID + 1].astype(BF16),
            "W4": W4.astype(BF16),
            "b1": b1.reshape(HID, 1).astype(np.float32),
            "b2": b2.reshape(HID, 1).astype(np.float32),
            "b3": b3.reshape(HID, 1).astype(np.float32),
            "b4": b4.reshape(1, 1).astype(np.float32),
        })

    res = run_bass_kernel_spmd(nc, in_maps, core_ids=list(range(cfg.CORES)),
                               trace=trace)

    E = edge_index.shape[1]
    out = np.zeros(E, np.float32)
    S_blk, goff, maxblk = layout["S_blk"], layout["goff"], layout["maxblk"]
    NQ = cfg.NQ
    for c in range(cfg.CORES):
        z = res.results[c]["zout"]  # [BLK, maxblk]
        orig = per_core[c]["slot_orig"]
        boff = 0
        for i in range(cfg.BLK):
            sb = int(S_blk[i])
            if sb == 0:
                continue
            o = orig[boff:boff + sb]
            m = o >= 0
            out[o[m]] = z[i, :sb][m]
            boff += sb
    return out, res


def kernel(x, edge_index, edge_attr, W1, b1, W2, b2, W3, b3, W4, b4):
    x = np.asarray(x)
    out, _ = _run(PROD, x, np.asarray(edge_index), np.asarray(edge_attr),
                  np.asarray(W1), np.asarray(b1), np.asarray(W2), np.asarray(b2),
                  np.asarray(W3), np.asarray(b3), np.asarray(W4), np.asarray(b4))
    return out

